# revision 6
# baseline (speedup 1.0000x reference)
"""Causal attention kernel for 8 Trainium2 NeuronCores, V-exchange variant.

Problem: x[4,2048,1024] fp32, Wq/Wk/Wv[1024,1024] fp32 (nn.Linear: y = x @ W.T),
single-head causal attention, softmax(QK^T/sqrt(D)) @ V.

Sharding: 2 cores per batch; queries split by row parity (core h takes global
rows s with s % 2 == h). Keys are PERMUTED per core: own-parity keys first
(slots 0..1023, ascending), partner-parity keys second (slots 1024..2047).
Under this order the core's queries ARE its first 1024 key slots, so the
query projection reads the same x^T slabs as K/V (no separate xq input), and
the causal structure vs. 128-slot key chunks is uniform across cores with all
h-dependence pushed into host-built additive mask tiles.

V dedup: each core computes V only for its own 1024 keys; the pair swaps
halves with ONE ReduceScatter(add) over [myV, myV] producing
Vsum[t] = V(own key t) + V(partner key t). The AV matmul uses
  sum_own p*V_own + sum_par p*V_par
    = sum (p_own - p_par)*V_own + sum p_par*Vsum,
so no un-mixing pass is needed: pdiff tiles (DVE sub of exp'd score tiles)
replace the own-P operand. This removes half the V projection FLOPs
(65536 PE columns) for one 2MB-output collective that overlaps projections.

All PSUM flows through ONE bank-shaped tile pool: TileContext emits a
cross-engine event-semaphore barrier at every PSUM pool close, which showed
up as ~0.7us PE stalls at each phase boundary; a single pool with 8 rotating
bank tiles keeps >=3 iterations of WAR slack in every phase and has no
boundaries. Other scheduling tricks: dummy matmuls pre-ramp the PE p-state
during the head DMAs; the V loop is do-outer so each weight chunk is consumed
as its DMA lands; AV runs denominators first and es-serial so reciprocal and
the es0 store drain under later matmuls; the last output drains in two column
halves on separate engines and DGE queues.

Device layout (PE matmul computes out = lhsT.T @ rhs over the 128-partition
contraction dim): host passes x^T (permuted) and W^T in bf16; all matmuls
accumulate in fp32 PSUM; scores are computed transposed St[k,q]; no
max-subtraction (logits bounded ~|2.5|); masked logits get -1e30 via mask
tiles; softmax denominator via ones-vector matmuls on the PE.
"""

import numpy as np

B, S, D, P = 4, 2048, 1024, 128
NQ = S // 2          # queries per core = own-parity keys
QT = 256             # score-tile width in (core-local) query dim
NEG = -1e30
N_CORES = 8
GROUPS = [[0, 1], [2, 3], [4, 5], [6, 7]]

_cache = {}


def _build():
    import concourse.mybir as mybir
    import concourse.tile as tile
    from concourse import bacc

    f32 = mybir.dt.float32
    bf = mybir.dt.bfloat16

    nc = bacc.Bacc(num_devices=N_CORES)

    xT = nc.dram_tensor("xT", [D, S], bf, kind="ExternalInput")
    wqT = nc.dram_tensor("wqT", [D, D], bf, kind="ExternalInput")
    wkT = nc.dram_tensor("wkT", [D, D], bf, kind="ExternalInput")
    wvT = nc.dram_tensor("wvT", [D, D], bf, kind="ExternalInput")
    masks = nc.dram_tensor("masks", [4, P, QT], f32, kind="ExternalInput")
    out = nc.dram_tensor("out", [NQ, D], f32, kind="ExternalOutput")

    xT3 = xT.ap().rearrange("(do di) s -> di do s", di=P)
    wq3 = wqT.ap().rearrange("(do di) e -> di do e", di=P)
    wk3 = wkT.ap().rearrange("(do di) e -> di do e", di=P)
    wv3 = wvT.ap().rearrange("(do di) e -> di do e", di=P)
    out_ap = out.ap()
    masks_ap = masks.ap()

    EXP = mybir.ActivationFunctionType.Exp
    COPYF = mybir.ActivationFunctionType.Copy
    SCALE = 1.0 / np.sqrt(np.float32(D))

    with tile.TileContext(nc) as tc:
        with (
            tc.tile_pool(name="const", bufs=1) as const_pool,
            tc.tile_pool(name="prod", bufs=1) as prod,
            tc.tile_pool(name="ins", bufs=1) as ins_pool,
            tc.tile_pool(name="wk", bufs=2) as wk_pool,
            tc.tile_pool(name="wq", bufs=2) as wq_pool,
            tc.tile_pool(name="dswap", bufs=1, space="DRAM") as dram_pool,
            tc.tile_pool(name="pt", bufs=3) as pt_pool,
            tc.tile_pool(name="ptd", bufs=3) as ptd_pool,
            tc.tile_pool(name="ob", bufs=3) as ob_pool,
            tc.tile_pool(name="rc", bufs=4) as rc_pool,
            tc.tile_pool(name="u", bufs=8, space="PSUM") as upool,
        ):
            def bank():
                return upool.tile([P, 512], f32, tag="u", name="u")

            # ---- head: V-projection deps lead; dummy matmuls pre-ramp PE ----
            dummy = const_pool.tile([P, 256], bf)
            nc.gpsimd.memset(dummy[:], 0.0)

            wv_sb = ins_pool.tile([P, 8, D], bf, name="wv_sb")
            xt_tiles = {}   # (slab, d0) -> (tile, dn)

            def xt_dma(slab, d0, dn):
                t = ins_pool.tile(
                    [P, dn, 512], bf, tag=f"xt{slab}d{d0}", name=f"xt{slab}d{d0}"
                )
                xt_tiles[(slab, d0)] = (t, dn)
                nc.sync.dma_start(
                    t[:], xT3[:, d0 : d0 + dn, slab * 512 : (slab + 1) * 512]
                )

            def xchunk(slab, do, cols):
                for (sl, d0), (t, dn) in xt_tiles.items():
                    if sl == slab and d0 <= do < d0 + dn:
                        return t[:, do - d0, cols]
                raise AssertionError((slab, do))

            # head DMA order: feed the do-outer V loop for slab 0 first; the
            # first pieces are halved so the first matmul starts ~1us sooner
            nc.sync.dma_start(wv_sb[:, 0, 0:512], wv3[:, 0, 0:512])
            xt_dma(0, 0, 1)
            nc.sync.dma_start(wv_sb[:, 0, 512:1024], wv3[:, 0, 512:1024])
            xt_dma(0, 1, 1)
            nc.sync.dma_start(wv_sb[:, 1, :], wv3[:, 1, :])
            xt_dma(0, 2, 2)
            nc.sync.dma_start(wv_sb[:, 2, :], wv3[:, 2, :])
            nc.sync.dma_start(wv_sb[:, 3, :], wv3[:, 3, :])
            xt_dma(0, 4, 4)
            for do in range(4, 8):
                nc.sync.dma_start(wv_sb[:, do, :], wv3[:, do, :])
            xt_dma(1, 0, 2)
            xt_dma(1, 2, 2)
            xt_dma(1, 4, 4)
            # wk for the first Kt phase, streamed behind the V-phase inputs
            wk0p = [
                wk_pool.tile([P, 2, 2 * P], bf, tag=f"wk0p{i}", name=f"wk0p{i}")
                for i in range(4)
            ]
            for i in range(4):
                nc.sync.dma_start(wk0p[i][:], wk3[:, 2 * i : 2 * i + 2, 0 : 2 * P])

            # preload the Exp activation table while DMAs land
            warm = const_pool.tile([P, 1], f32)
            nc.vector.memset(warm[:], 0.0)
            nc.scalar.activation(out=warm[:], in_=warm[:], func=EXP, scale=1.0)

            # PE pre-ramp: ~3us of throwaway matmuls so the p-state reaches
            # full speed before the first real matmul
            wu = bank()
            for i in range(14):
                nc.tensor.matmul(
                    wu[:, 0:256], dummy[:, 0:128], dummy[:],
                    start=(i == 0), stop=(i == 13),
                )

            # persistent per-core products
            kt_slabs = [prod.tile([P, 8, 512], bf, tag=f"kt{s}", name=f"kt{s}") for s in range(4)]
            v_own = [prod.tile([P, 4, D], bf, tag=f"v{s}", name=f"v{s}") for s in range(2)]
            v_rec = [prod.tile([P, 4, D], bf, tag=f"vr{s}", name=f"vr{s}") for s in range(2)]
            qt = prod.tile([P, 8, NQ], bf, tag="qt")

            vswap = dram_pool.tile([2, 2, P, 4, D], bf)
            vrecv = dram_pool.tile([2, P, 4, D], bf)

            deferred_loads = {}

            def score_chunk(a, b, c, pts, ptd):
                own = c < 8
                cc = c % 8
                active = [q for q in (a, b) if cc < 2 * q + 2]
                # separate PSUM bank per block: interleaving two accumulation
                # sequences into one bank corrupts PSUM on hardware
                pss_t = {q: bank() for q in active}
                off = {a: 0, b: 0}
                # the second diagonal chunk is fully masked for the first 128
                # queries of the block -> skip those columns entirely
                j0s = {q: 128 if cc == 2 * q + 1 else 0 for q in active}
                for ec in range(8):
                    for q in active:
                        j0 = j0s[q]
                        nc.tensor.matmul(
                            pss_t[q][:, off[q] + j0 : off[q] + QT],
                            kt_slabs[c // 4][:, ec, (c % 4) * P : (c % 4 + 1) * P],
                            qt[:, ec, q * QT + j0 : (q + 1) * QT],
                            start=(ec == 0),
                            stop=(ec == 7),
                        )
                for q in active:
                    m2 = cc - 2 * q
                    j0 = j0s[q]
                    sl = slice(off[q] + j0, off[q] + QT)
                    if m2 >= 0:
                        nc.vector.tensor_add(
                            out=pss_t[q][:, sl],
                            in0=pss_t[q][:, sl],
                            in1=deferred_loads["mask"][
                                :, (0 if own else 2) + m2, j0:QT
                            ],
                        )
                    nc.scalar.activation(
                        out=pts[q][:, c, j0:QT],
                        in_=pss_t[q][:, sl],
                        func=EXP,
                        scale=SCALE,
                    )
                    if not own:
                        nc.vector.tensor_sub(
                            out=ptd[q][:, cc, j0:QT],
                            in0=pts[q][:, cc, j0:QT],
                            in1=pts[q][:, c, j0:QT],
                        )

            # ---- V projection, own keys only (slabs 0,1) ----
            # slab 0: do-outer (each wv/x chunk consumed as its DMA lands,
            # 8 PSUM banks open); slab 1: kq-outer (data resident by then)
            pvs = [bank() for _ in range(8)]
            for do in range(8):
                for es in range(2):
                    for kq in range(4):
                        nc.tensor.matmul(
                            pvs[kq * 2 + es][:],
                            xchunk(0, do, slice(kq * P, (kq + 1) * P)),
                            wv_sb[:, do, es * 512 : (es + 1) * 512],
                            start=(do == 0),
                            stop=(do == 7),
                        )
            for kq in range(4):
                nc.vector.tensor_copy(
                    out=v_own[0][:, kq, 0:512], in_=pvs[kq * 2][:]
                )
                nc.scalar.copy(
                    out=v_own[0][:, kq, 512:1024], in_=pvs[kq * 2 + 1][:]
                )
            nc.gpsimd.dma_start(vswap[0, 0], v_own[0][:])
            nc.gpsimd.dma_start(vswap[1, 0], v_own[0][:])
            for kq in range(4):
                pv2 = [bank() for _ in range(2)]
                for do in range(8):
                    for es in range(2):
                        nc.tensor.matmul(
                            pv2[es][:],
                            xchunk(1, do, slice(kq * P, (kq + 1) * P)),
                            wv_sb[:, do, es * 512 : (es + 1) * 512],
                            start=(do == 0),
                            stop=(do == 7),
                        )
                nc.vector.tensor_copy(
                    out=v_own[1][:, kq, 0:512], in_=pv2[0][:]
                )
                nc.scalar.copy(
                    out=v_own[1][:, kq, 512:1024], in_=pv2[1][:]
                )
            # both RS slots get my V half: out = V_own + V_partner
            nc.gpsimd.dma_start(vswap[0, 1], v_own[1][:])
            nc.gpsimd.dma_start(vswap[1, 1], v_own[1][:])

            nc.gpsimd.collective_compute(
                "ReduceScatter",
                mybir.AluOpType.add,
                replica_groups=GROUPS,
                ins=[vswap.opt()],
                outs=[vrecv.opt()],
            )
            # receive Vsum; on the gpsimd queue so the long collective wait
            # never head-blocks the sync DMA stream
            for s2 in range(2):
                nc.gpsimd.dma_start(v_rec[s2][:], vrecv[s2])

            def kt_iter(half, ec, wkchunk):
                pss = [bank() for _ in range(2)]
                for do in range(8):
                    for s2 in range(2):
                        e2 = ec % 2
                        nc.tensor.matmul(
                            pss[s2][:],
                            wkchunk(do, slice(e2 * P, (e2 + 1) * P)),
                            xchunk(half * 2 + s2, do, slice(None)),
                            start=(do == 0),
                            stop=(do == 7),
                        )
                for s2 in range(2):
                    if s2 == 0:
                        nc.vector.tensor_copy(
                            out=kt_slabs[half * 2 + s2][:, ec, :],
                            in_=pss[s2][:],
                        )
                    else:
                        nc.scalar.copy(
                            out=kt_slabs[half * 2 + s2][:, ec, :],
                            in_=pss[s2][:],
                        )

            def kt_half(half, first_wk=None, emit_after=()):
                for pc in range(4):
                    if pc == 0 and first_wk is not None:
                        lo_hi = first_wk

                        if isinstance(lo_hi, list):
                            def wkchunk(do, esl, parts=lo_hi):
                                return parts[do // 2][:, do % 2, esl]
                        else:
                            def wkchunk(do, esl, wk_pair=lo_hi):
                                return wk_pair[:, do, esl]
                    else:
                        wk_pair = wk_pool.tile([P, 8, 2 * P], bf, tag="wk")
                        nc.sync.dma_start(
                            wk_pair[:], wk3[:, :, pc * 2 * P : (pc + 1) * 2 * P]
                        )

                        def wkchunk(do, esl, wk_pair=wk_pair):
                            return wk_pair[:, do, esl]
                    for e2 in range(2):
                        ec = pc * 2 + e2
                        kt_iter(half, ec, wkchunk)
                        for when, what in emit_after:
                            if when == ec:
                                what()

            def emit_wq0():
                t = wq_pool.tile([P, 8, 2 * P], bf, tag="wq", name="wq0")
                nc.sync.dma_start(t[:], wq3[:, :, 0 : 2 * P])
                deferred_loads["wq0"] = t

            def emit_masks():
                mask_sb = const_pool.tile([P, 4, QT], f32)
                nc.sync.dma_start(
                    mask_sb[:], masks_ap.rearrange("m p j -> p m j")
                )
                ones_sb = const_pool.tile([P, 1], bf)
                nc.vector.memset(ones_sb[:], 1.0)
                deferred_loads["mask"] = mask_sb
                deferred_loads["ones"] = ones_sb

            # ---- K^T own-key slabs 0,1 ----
            kt_half(0, first_wk=wk0p,
                    emit_after=((5, emit_wq0), (6, emit_masks)))

            # ---- Q^T projection: rhs is the own-key x slabs 0,1 ----
            wk1_first = None
            for pc in range(4):
                if pc == 0:
                    wq_pair = deferred_loads["wq0"]
                else:
                    wq_pair = wq_pool.tile([P, 8, 2 * P], bf, tag="wq")
                    nc.sync.dma_start(
                        wq_pair[:], wq3[:, :, pc * 2 * P : (pc + 1) * 2 * P]
                    )
                if pc == 1:
                    # partner-key x slabs stream in for the last Kt phase
                    xt_dma(2, 0, 4)
                    xt_dma(2, 4, 4)
                if pc == 2:
                    xt_dma(3, 0, 4)
                    xt_dma(3, 4, 4)
                for e2 in range(2):
                    ec = pc * 2 + e2
                    pqs = [bank() for _ in range(2)]
                    for do in range(8):
                        for qs in range(2):
                            nc.tensor.matmul(
                                pqs[qs][:],
                                wq_pair[:, do, e2 * P : (e2 + 1) * P],
                                xchunk(qs, do, slice(None)),
                                start=(do == 0),
                                stop=(do == 7),
                            )
                    nc.vector.tensor_copy(
                        out=qt[:, ec, 0:512], in_=pqs[0][:]
                    )
                    nc.scalar.copy(
                        out=qt[:, ec, 512:1024], in_=pqs[1][:]
                    )
                    if ec == 5:
                        wk1_first = wk_pool.tile(
                            [P, 8, 2 * P], bf, tag="wk", name="wk1f"
                        )
                        nc.sync.dma_start(wk1_first[:], wk3[:, :, 0 : 2 * P])

            # ---- K^T partner-key slabs 2,3 ----
            kt_half(1, first_wk=wk1_first)

            # ---- attention: per 256-query block pair: scores over own then
            #      partner key chunks, exp, pdiff = p_own - p_par, then AV as
            #      pdiff@V_own + p_par@Vsum ----
            for a, b in ((0, 1), (2, 3)):
                pts = {
                    q: pt_pool.tile([P, 16, QT], bf, tag="pt", name=f"pt{q}")
                    for q in (a, b)
                }
                ptd = {
                    q: ptd_pool.tile([P, 8, QT], bf, tag="ptd", name=f"ptd{q}")
                    for q in (a, b)
                }
                chunk_seq = list(range(0, 2 * b + 2)) + list(
                    range(8, 8 + 2 * b + 2)
                )
                for c in chunk_seq:
                    score_chunk(a, b, c, pts, ptd)
                for q in (a, b):
                    for qc in range(2):
                        npair = 2 * q + 1 + qc
                        qsl = slice(qc * P, (qc + 1) * P)
                        last_grp = b == 3 and q == b and qc == 1
                        dnb = bank()
                        dn = dnb[:, 0:1]
                        pos = [bank() for _ in range(1 if last_grp else 2)]
                        # denominators first so the reciprocal overlaps the
                        # AV matmuls
                        for cc in range(npair):
                            nc.tensor.matmul(
                                dn, pts[q][:, cc, qsl],
                                deferred_loads["ones"][:],
                                start=cc == 0, stop=False,
                            )
                            nc.tensor.matmul(
                                dn, pts[q][:, 8 + cc, qsl],
                                deferred_loads["ones"][:],
                                start=False, stop=cc == npair - 1,
                            )
                        rc = rc_pool.tile([P, 1], f32, tag="rc")
                        nc.vector.reciprocal(out=rc[:], in_=dn)
                        q0 = q * QT + qc * P
                        # es-serial: es0's scale+store drains while es1 still
                        # accumulates on the PE
                        for es in range(2):
                            esl = slice(es * 512, (es + 1) * 512)
                            if last_grp and es == 1:
                                # final output: accumulate + drain in two
                                # column pieces, each in its OWN psum bank
                                # (avoids tile-level false sharing) and on
                                # separate engines/DGE queues so the
                                # post-matmul tail is short
                                for hf, (c0, cw) in enumerate(
                                    ((0, 384), (384, 128))
                                ):
                                    pob = bank()
                                    for cc in range(npair):
                                        s, kq = cc // 4, cc % 4
                                        vsl = slice(512 + c0, 512 + c0 + cw)
                                        nc.tensor.matmul(
                                            pob[:, 0:cw],
                                            ptd[q][:, cc, qsl],
                                            v_own[s][:, kq, vsl],
                                            start=cc == 0, stop=False,
                                        )
                                        nc.tensor.matmul(
                                            pob[:, 0:cw],
                                            pts[q][:, 8 + cc, qsl],
                                            v_rec[s][:, kq, vsl],
                                            start=False,
                                            stop=cc == npair - 1,
                                        )
                                    obh = ob_pool.tile(
                                        [P, cw], f32, tag=f"obh{hf}"
                                    )
                                    if hf == 0:
                                        nc.scalar.activation(
                                            out=obh[:], in_=pob[:, 0:cw],
                                            func=COPYF, scale=rc[:],
                                        )
                                        nc.scalar.dma_start(
                                            out_ap[q0 : q0 + P, 512:896],
                                            obh[:],
                                        )
                                    else:
                                        nc.vector.tensor_scalar_mul(
                                            out=obh[:],
                                            in0=pob[:, 0:cw],
                                            scalar1=rc[:],
                                        )
                                        nc.sync.dma_start(
                                            out_ap[q0 : q0 + P, 896:1024],
                                            obh[:],
                                        )
                                continue
                            for cc in range(npair):
                                s, kq = cc // 4, cc % 4
                                nc.tensor.matmul(
                                    pos[es][:], ptd[q][:, cc, qsl],
                                    v_own[s][:, kq, esl],
                                    start=cc == 0, stop=False,
                                )
                                nc.tensor.matmul(
                                    pos[es][:], pts[q][:, 8 + cc, qsl],
                                    v_rec[s][:, kq, esl],
                                    start=False, stop=cc == npair - 1,
                                )
                            ob = ob_pool.tile([P, 512], f32, tag="ob")
                            if es == 0:
                                nc.vector.tensor_scalar_mul(
                                    out=ob[:], in0=pos[es][:], scalar1=rc[:]
                                )
                            else:
                                nc.scalar.activation(
                                    out=ob[:], in_=pos[es][:],
                                    func=COPYF, scale=rc[:],
                                )
                            nc.sync.dma_start(
                                out_ap[q0 : q0 + P, esl], ob[:]
                            )

    nc.compile()
    return nc


def _get_nc():
    if "nc" not in _cache:
        _cache["nc"] = _build()
    return _cache["nc"]


def _host_masks(h: int) -> np.ndarray:
    # mask[m, p, j]: additive tile for score chunk at key-chunk offset
    # m2 = m % 2 relative to query block (m<2: own-parity keys, m>=2:
    # partner-parity). Own: keep iff p <= j - 128*m2; partner: keep iff
    # p <= j - 128*m2 - (1-h).
    p = np.arange(P)[:, None]
    j = np.arange(QT)[None, :]
    tiles = []
    for base in (0, 1 - h):
        for m2 in (0, 1):
            keep = p <= j - 128 * m2 - base
            tiles.append(np.where(keep, np.float32(0.0), np.float32(NEG)))
    return np.stack(tiles).astype(np.float32)


def _perm(h: int) -> np.ndarray:
    return np.concatenate([np.arange(h, S, 2), np.arange(1 - h, S, 2)])


def make_in_maps(x, Wq, Wk, Wv):
    import ml_dtypes

    bf = ml_dtypes.bfloat16
    wqT = np.ascontiguousarray(Wq.T).astype(bf)
    wkT = np.ascontiguousarray(Wk.T).astype(bf)
    wvT = np.ascontiguousarray(Wv.T).astype(bf)
    masks_h = [_host_masks(0), _host_masks(1)]
    in_maps = []
    for c in range(N_CORES):
        b, h = c // 2, c % 2
        xb = np.asarray(x[b], dtype=np.float32)[_perm(h)]
        in_maps.append(
            {
                "xT": np.ascontiguousarray(xb.T).astype(bf),
                "wqT": wqT,
                "wkT": wkT,
                "wvT": wvT,
                "masks": masks_h[h],
            }
        )
    return in_maps


def kernel(x, Wq, Wk, Wv):
    from concourse.bass_utils import run_bass_kernel_spmd

    nc = _get_nc()
    in_maps = make_in_maps(x, Wq, Wk, Wv)
    res = run_bass_kernel_spmd(nc, in_maps, core_ids=list(range(N_CORES)))
    out = np.empty((B, S, D), dtype=np.float32)
    for c in range(N_CORES):
        b, h = c // 2, c % 2
        out[b, h::2, :] = res.results[c]["out"]
    return out
